# revision 3
# baseline (speedup 1.0000x reference)
"""Causal multi-head self-attention (B=2, S=2048, D=1024, H=16) for 8 trn2
NeuronCores.

Sharding: 2-way data parallel over batch x 4-way tensor parallel over heads.
Core c handles batch c//4 and heads [(c%4)*4, (c%4)*4+4) (dh_local=256).
Each core computes its heads' q/k/v projections, causal softmax attention
probabilities (written as the p_attn output shard), the attention context,
and a partial output projection over its 256 head-dims. The host sums the 4
partial output projections per batch (the TP all-reduce) and adds bo.

The upper-triangle (masked) region of p_attn is never written on device: the
runtime hands the kernel zero-initialised output buffers (both the native
run_neff path and the bass2jax/PJRT donation path guarantee this), so the
strictly-causal zeros come for free.

Device-side layout notes:
  - activations are passed pre-transposed (xT: [D, S]) so every matmul
    contraction dim lands on SBUF partitions.
  - q/k/v are produced transposed ([dh, S]); v is then PE-transposed back to
    [S, dh] (needed as the moving operand of p @ v).
  - softmax needs no running-max: scores/sqrt(D) have |s| < ~1, exp is safe.
"""

import os
import numpy as np

B, S, D, H = 2, 2048, 1024, 16
N_CORES = 8
TP = 4              # cores per batch (head-parallel group size)
HL = H // TP        # heads per core = 4
DK = D // H         # 64
DHL = HL * DK       # local head dims per core = 256
P = 128             # SBUF partitions
KO = D // P         # 8 contraction subtiles for the projections
CW = 512            # chunk width (matmul moving-dim / PSUM bank)
NEG = -30000.0      # additive causal mask (exp -> exact 0 after /32 scale)
SCALE = 1.0 / np.sqrt(np.float32(D))  # faithful-to-source sqrt(d_model) scale

# Per-matmul-site float32r toggles (fp32r streams 1 col/cycle vs fp32's 4
# when the moving dim is >=256; numerics are slightly relaxed).
F32R_PROJ = bool(int(os.environ.get("KERNEL_F32R", "0")))
F32R_SCORES = F32R_PROJ
F32R_PV = False      # pv moving dim is 64 -> no fp32r win; keep exact
F32R_OUT = F32R_PROJ

_CACHE = {}


def _build_nc():
    import concourse.bass as bass
    import concourse.mybir as mybir
    import concourse.tile as tile
    from concourse import bacc

    f32 = mybir.dt.float32
    f32r = mybir.dt.float32r
    Act = mybir.ActivationFunctionType

    nc = bacc.Bacc(
        "TRN2",
        target_bir_lowering=False,
        debug=False,
        enable_asserts=False,
        num_devices=N_CORES,
    )

    # Per-core inputs.
    xqT = nc.dram_tensor("xqT", [D, S], f32, kind="ExternalInput").ap()
    xkT = nc.dram_tensor("xkT", [D, S], f32, kind="ExternalInput").ap()
    xvT = nc.dram_tensor("xvT", [D, S], f32, kind="ExternalInput").ap()
    wqT = nc.dram_tensor("wqT", [D, DHL], f32, kind="ExternalInput").ap()
    wkT = nc.dram_tensor("wkT", [D, DHL], f32, kind="ExternalInput").ap()
    wvT = nc.dram_tensor("wvT", [D, DHL], f32, kind="ExternalInput").ap()
    woT = nc.dram_tensor("woT", [DHL, D], f32, kind="ExternalInput").ap()
    bqkv = nc.dram_tensor("bqkv", [3, DHL], f32, kind="ExternalInput").ap()
    masks = nc.dram_tensor("masks", [4, P, CW], f32, kind="ExternalInput").ap()
    ident = nc.dram_tensor("ident", [P, P], f32, kind="ExternalInput").ap()

    # Per-core outputs.
    p_out = nc.dram_tensor("p_out", [HL, S, S], f32, kind="ExternalOutput").ap()
    o_out = nc.dram_tensor("o_out", [S, D], f32, kind="ExternalOutput").ap()

    def mm_dt(ap, use_f32r):
        return ap.bitcast(f32r) if use_f32r else ap

    with tile.TileContext(nc) as tc:
        with (
            tc.tile_pool(name="consts", bufs=1) as consts,
            tc.tile_pool(name="xin", bufs=2) as xin,
            tc.tile_pool(name="big", bufs=1) as big,
            tc.tile_pool(name="vt", bufs=2) as vtp,
            tc.tile_pool(name="pstrip", bufs=2) as pstrip,
            tc.tile_pool(name="ptile", bufs=3) as ptile,
            tc.tile_pool(name="outp", bufs=2) as outp,
            tc.tile_pool(name="small", bufs=4) as small,
            tc.tile_pool(name="ps512", bufs=2, space="PSUM") as ps512,
            tc.tile_pool(name="ps128", bufs=2, space="PSUM") as ps128,
            tc.tile_pool(name="psctx", bufs=2, space="PSUM") as psctx,
        ):
            # ---- constants -------------------------------------------------
            ident_sb = consts.tile([P, P], f32, tag="ident")
            nc.sync.dma_start(ident_sb[:], ident)
            masks_sb = consts.tile([P, 4, CW], f32, tag="masks")
            nc.sync.dma_start(masks_sb[:], masks.rearrange("m p c -> p m c"))
            wq_sb = consts.tile([P, KO, DHL], f32, tag="wq")
            nc.sync.dma_start(wq_sb[:], wqT.rearrange("(ko ki) m -> ki ko m", ki=P))
            wk_sb = consts.tile([P, KO, DHL], f32, tag="wk")
            nc.sync.dma_start(wk_sb[:], wkT.rearrange("(ko ki) m -> ki ko m", ki=P))
            wv_sb = consts.tile([P, KO, DHL], f32, tag="wv")
            nc.sync.dma_start(wv_sb[:], wvT.rearrange("(ko ki) m -> ki ko m", ki=P))
            wo_sb = consts.tile([P, DHL // P, D], f32, tag="wo")
            nc.sync.dma_start(wo_sb[:], woT.rearrange("(ko ki) n -> ki ko n", ki=P))
            bias_sb = consts.tile([P, 6], f32, tag="bias")
            nc.sync.dma_start(bias_sb[:], bqkv.rearrange("t (o p) -> p (t o)", p=P))

            # ---- phase P: projections -------------------------------------
            # qT/kT: [dh, S] with dh split into 2 partition blocks.
            qT_sb = big.tile([P, 2, S], f32, tag="qT")
            kT_sb = big.tile([P, 2, S], f32, tag="kT")
            # v: [S, dh] (s split into 16 partition blocks) for the p@v rhs.
            v_sb = big.tile([P, S // P, DHL], f32, tag="v")

            for t, (xT, w_sb) in enumerate(((xqT, wq_sb), (xkT, wk_sb), (xvT, wv_sb))):
                xT_r = xT.rearrange("(ko ki) s -> ki ko s", ki=P)
                for nch in range(S // CW):
                    x_t = xin.tile([P, KO, CW], f32, tag="x")
                    nc.sync.dma_start(x_t[:], xT_r[:, :, nch * CW:(nch + 1) * CW])
                    if t < 2:
                        dst = qT_sb if t == 0 else kT_sb
                        for mb in range(2):
                            ps = ps512.tile([P, CW], f32, tag="ps")
                            for ko in range(KO):
                                nc.tensor.matmul(
                                    ps[:],
                                    mm_dt(w_sb[:, ko, mb * P:(mb + 1) * P], F32R_PROJ),
                                    mm_dt(x_t[:, ko, :], F32R_PROJ),
                                    start=(ko == 0),
                                    stop=(ko == KO - 1),
                                )
                            nc.scalar.activation(
                                dst[:, mb, nch * CW:(nch + 1) * CW], ps[:],
                                Act.Identity, bias=bias_sb[:, t * 2 + mb:t * 2 + mb + 1],
                            )
                    else:
                        v_t = vtp.tile([P, 2, CW], f32, tag="vt")
                        for mb in range(2):
                            ps = ps512.tile([P, CW], f32, tag="ps")
                            for ko in range(KO):
                                nc.tensor.matmul(
                                    ps[:],
                                    mm_dt(w_sb[:, ko, mb * P:(mb + 1) * P], F32R_PROJ),
                                    mm_dt(x_t[:, ko, :], F32R_PROJ),
                                    start=(ko == 0),
                                    stop=(ko == KO - 1),
                                )
                            nc.scalar.activation(
                                v_t[:, mb, :], ps[:],
                                Act.Identity, bias=bias_sb[:, 4 + mb:5 + mb],
                            )
                        # vT chunk -> v (PE transpose via identity).
                        for j in range(CW // P):
                            sb = nch * (CW // P) + j
                            for mb in range(2):
                                pst = ps128.tile([P, P], f32, tag="pst")
                                nc.tensor.transpose(
                                    pst[:], v_t[:, mb, j * P:(j + 1) * P], ident_sb[:]
                                )
                                nc.scalar.copy(
                                    v_sb[:, sb, mb * P:(mb + 1) * P], pst[:]
                                )

            # ---- phase A: attention per local head ------------------------
            ctx_sb = big.tile([P, S // P, DHL], f32, tag="ctx")
            for h in range(HL):
                mb, off = h // 2, (h % 2) * DK
                for qb in range(S // P):
                    ncch = qb // 4 + 1        # 512-wide key chunks this strip
                    p_t = pstrip.tile([P, S], f32, tag="p")
                    sums_t = small.tile([P, 8], f32, tag="sums")
                    for c in range(ncch):
                        ps = ps512.tile([P, CW], f32, tag="ps")
                        nc.tensor.matmul(
                            ps[:],
                            mm_dt(qT_sb[off:off + DK, mb, qb * P:(qb + 1) * P], F32R_SCORES),
                            mm_dt(kT_sb[off:off + DK, mb, c * CW:(c + 1) * CW], F32R_SCORES),
                            start=True, stop=True,
                        )
                        if c == ncch - 1:
                            nc.vector.tensor_add(ps[:], ps[:], masks_sb[:, qb % 4, :])
                        nc.scalar.activation(
                            p_t[:, c * CW:(c + 1) * CW], ps[:], Act.Exp,
                            scale=float(SCALE), accum_out=sums_t[:, c:c + 1],
                        )
                    nc.vector.reduce_sum(
                        out=sums_t[:, 7:8], in_=sums_t[:, 0:ncch],
                        axis=mybir.AxisListType.X,
                    )
                    nc.vector.reciprocal(sums_t[:, 6:7], sums_t[:, 7:8])
                    nc.vector.tensor_scalar_mul(
                        p_t[:, 0:ncch * CW], p_t[:, 0:ncch * CW], sums_t[:, 6:7]
                    )
                    nc.sync.dma_start(
                        p_out[h, qb * P:(qb + 1) * P, 0:ncch * CW],
                        p_t[:, 0:ncch * CW],
                    )
                    # p @ v for this strip: transpose p blocks, accumulate ctx.
                    psc = psctx.tile([P, DK], f32, tag="psc")
                    for kb in range(qb + 1):
                        pst = ps128.tile([P, P], f32, tag="pst")
                        nc.tensor.transpose(
                            pst[:], p_t[:, kb * P:(kb + 1) * P], ident_sb[:]
                        )
                        pT_t = ptile.tile([P, P], f32, tag="pT")
                        nc.vector.tensor_copy(pT_t[:], pst[:])
                        nc.tensor.matmul(
                            psc[:],
                            mm_dt(pT_t[:], F32R_PV),
                            mm_dt(v_sb[:, kb, h * DK:(h + 1) * DK], F32R_PV),
                            start=(kb == 0), stop=(kb == qb),
                        )
                    nc.scalar.copy(ctx_sb[:, qb, h * DK:(h + 1) * DK], psc[:])

            # ---- phase O: output projection (partial over local dh) -------
            ctxT_sb = big.tile([P, DHL // P, S], f32, tag="ctxT")
            for qb in range(S // P):
                for mbd in range(DHL // P):
                    pst = ps128.tile([P, P], f32, tag="pst")
                    nc.tensor.transpose(
                        pst[:], ctx_sb[:, qb, mbd * P:(mbd + 1) * P], ident_sb[:]
                    )
                    nc.vector.tensor_copy(
                        ctxT_sb[:, mbd, qb * P:(qb + 1) * P], pst[:]
                    )
            for sb in range(S // P):
                o_t = outp.tile([P, D], f32, tag="o")
                for nh in range(D // CW):
                    ps = ps512.tile([P, CW], f32, tag="ps")
                    for mbd in range(DHL // P):
                        nc.tensor.matmul(
                            ps[:],
                            mm_dt(ctxT_sb[:, mbd, sb * P:(sb + 1) * P], F32R_OUT),
                            mm_dt(wo_sb[:, mbd, nh * CW:(nh + 1) * CW], F32R_OUT),
                            start=(mbd == 0), stop=(mbd == DHL // P - 1),
                        )
                    nc.scalar.copy(o_t[:, nh * CW:(nh + 1) * CW], ps[:])
                nc.sync.dma_start(o_out[sb * P:(sb + 1) * P, :], o_t[:])

    nc.compile()
    return nc


def _get_nc():
    if "nc" not in _CACHE:
        _CACHE["nc"] = _build_nc()
    return _CACHE["nc"]


def _host_masks():
    # masks[m][qr, j]: additive causal mask for the diagonal 512-wide chunk
    # when the strip's diag block sits at key-offset m*128 within the chunk.
    m = np.full((4, P, CW), NEG, np.float32)
    for v in range(4):
        qr = np.arange(P)[:, None]
        j = np.arange(CW)[None, :]
        m[v][j <= v * P + qr] = 0.0
    return m


def kernel(query, key, value, Wq, bq, Wk, bk, Wv, bv, Wo, bo):
    from concourse.bass_utils import run_bass_kernel_spmd

    nc = _get_nc()

    query = np.asarray(query, np.float32)
    key = np.asarray(key, np.float32)
    value = np.asarray(value, np.float32)
    WqT = np.ascontiguousarray(np.asarray(Wq, np.float32).T)
    WkT = np.ascontiguousarray(np.asarray(Wk, np.float32).T)
    WvT = np.ascontiguousarray(np.asarray(Wv, np.float32).T)
    WoT = np.ascontiguousarray(np.asarray(Wo, np.float32).T)
    bq = np.asarray(bq, np.float32)
    bk = np.asarray(bk, np.float32)
    bv = np.asarray(bv, np.float32)
    bo = np.asarray(bo, np.float32)

    masks = _host_masks()
    ident = np.eye(P, dtype=np.float32)

    in_maps = []
    for c in range(N_CORES):
        b, g = c // TP, c % TP
        cols = slice(g * DHL, (g + 1) * DHL)   # head dims owned by this core
        in_maps.append({
            "xqT": np.ascontiguousarray(query[b].T),
            "xkT": np.ascontiguousarray(key[b].T),
            "xvT": np.ascontiguousarray(value[b].T),
            "wqT": np.ascontiguousarray(WqT[:, cols]),
            "wkT": np.ascontiguousarray(WkT[:, cols]),
            "wvT": np.ascontiguousarray(WvT[:, cols]),
            "woT": np.ascontiguousarray(WoT[cols, :]),
            "bqkv": np.ascontiguousarray(np.stack([bq[cols], bk[cols], bv[cols]])),
            "masks": masks,
            "ident": ident,
        })

    res = run_bass_kernel_spmd(
        nc, in_maps, core_ids=list(range(N_CORES)),
        trace=bool(int(os.environ.get("KERNEL_TRACE", "0"))),
    )
    _CACHE["last_result"] = res

    p_attn = np.empty((B, H, S, S), np.float32)
    out = np.empty((B, S, D), np.float32)
    for b in range(B):
        acc = None
        for g in range(TP):
            r = res.results[b * TP + g]
            p_attn[b, g * HL:(g + 1) * HL] = r["p_out"]
            acc = r["o_out"] if acc is None else acc + r["o_out"]
        out[b] = acc + bo
    return out, p_attn


# revision 7
# speedup vs baseline: 2.1086x; 2.1086x over previous
"""Causal multi-head self-attention (B=2, S=2048, D=1024, H=16) for 8 trn2
NeuronCores.

Sharding: 2-way data parallel over batch x 4-way tensor parallel over heads.
Core c handles batch c//4 and heads [(c%4)*4, (c%4)*4+4) (dh_local=256).
Each core computes its heads' q/k/v projections, causal softmax attention
probabilities (written as the p_attn output shard), the attention context,
and a partial output projection over its 256 head-dims. The host sums the 4
partial output projections per batch (the TP all-reduce) and adds bo.

The upper-triangle (masked) region of p_attn is never written on device: the
runtime hands the kernel zero-initialised output buffers (both the native
run_neff path and the bass2jax/PJRT donation path guarantee this), so the
strictly-causal zeros come for free.

Device-side notes:
  - activations arrive pre-transposed (xT: [D, S]) so every matmul
    contraction dim lands on SBUF partitions.
  - q/k/v are produced transposed ([dh, S]); v is then PE-transposed back to
    [S, dh] (needed as the moving operand of p @ v) and stored fp16.
  - softmax needs no running-max pass: scores/sqrt(D) have |s| < ~1.
  - scores/projection/output matmuls run as float32r (full-rate PE); the
    p @ v matmul (moving dim 64, no fp32r win) runs in fp16. p_attn itself
    is computed and written in exact fp32.
"""

import os
import numpy as np

B, S, D, H = 2, 2048, 1024, 16
N_CORES = 8
TP = 4              # cores per batch (head-parallel group size)
HL = H // TP        # heads per core = 4
DK = D // H         # 64
DHL = HL * DK       # local head dims per core = 256
P = 128             # SBUF partitions
KO = D // P         # 8 contraction subtiles for the projections
CW = 512            # max chunk width (matmul moving-dim / PSUM bank)
NEG = -30000.0      # additive causal mask (exp -> exact 0 after /32 scale)
SCALE = 1.0 / float(np.sqrt(np.float32(D)))  # faithful sqrt(d_model) scale

F32R = bool(int(os.environ.get("KERNEL_F32R", "1")))
PV_F16 = bool(int(os.environ.get("KERNEL_PV_F16", "1")))

_CACHE = {}


def _build_nc():
    import concourse.bass as bass
    import concourse.mybir as mybir
    import concourse.tile as tile
    from concourse import bacc

    f32 = mybir.dt.float32
    f32r = mybir.dt.float32r
    f16 = mybir.dt.float16
    pv_dt = f16 if PV_F16 else f32
    mmdt = f32r if F32R else f32   # dtype of tensors feeding PE matmuls
    Act = mybir.ActivationFunctionType

    nc = bacc.Bacc(
        "TRN2",
        target_bir_lowering=False,
        debug=False,
        enable_asserts=False,
        num_devices=N_CORES,
    )

    # Per-core inputs.
    xqT = nc.dram_tensor("xqT", [D, S], mmdt, kind="ExternalInput").ap()
    xkT = nc.dram_tensor("xkT", [D, S], mmdt, kind="ExternalInput").ap()
    xvT = nc.dram_tensor("xvT", [D, S], mmdt, kind="ExternalInput").ap()
    wqT = nc.dram_tensor("wqT", [D, DHL], mmdt, kind="ExternalInput").ap()
    wkT = nc.dram_tensor("wkT", [D, DHL], mmdt, kind="ExternalInput").ap()
    wvT = nc.dram_tensor("wvT", [D, DHL], mmdt, kind="ExternalInput").ap()
    woT = nc.dram_tensor("woT", [DHL, D], mmdt, kind="ExternalInput").ap()
    bqkv = nc.dram_tensor("bqkv", [3, DHL], f32, kind="ExternalInput").ap()
    mask = nc.dram_tensor("mask", [P, P], f32, kind="ExternalInput").ap()
    ident = nc.dram_tensor("ident", [P, P], f32, kind="ExternalInput").ap()

    # Per-core outputs.
    p_out = nc.dram_tensor("p_out", [HL, S, S], f32, kind="ExternalOutput").ap()
    o_out = nc.dram_tensor("o_out", [S, D], f32, kind="ExternalOutput").ap()

    def tr(out_ap, in_ap, ident_ap):
        nc.tensor.transpose(out_ap, in_ap, ident_ap)

    with tile.TileContext(nc) as tc:
        with (
            tc.tile_pool(name="consts", bufs=1) as consts,
            tc.tile_pool(name="xin", bufs=2) as xin,
            tc.tile_pool(name="big", bufs=1) as big,
            tc.tile_pool(name="vt", bufs=2) as vtp,
            tc.tile_pool(name="pstrip", bufs=2) as pstrip,
            tc.tile_pool(name="ptile", bufs=3) as ptile,
            tc.tile_pool(name="outp", bufs=2) as outp,
            tc.tile_pool(name="small", bufs=4) as small,
            tc.tile_pool(name="ps512", bufs=3, space="PSUM") as ps512,
            tc.tile_pool(name="pstr", bufs=2, space="PSUM") as pstr,
            tc.tile_pool(name="psctx", bufs=2, space="PSUM") as psctx,
        ):
            # ---- constants -------------------------------------------------
            ident_sb = consts.tile([P, P], f32, tag="ident")
            nc.sync.dma_start(ident_sb[:], ident)
            mask_sb = consts.tile([P, P], f32, tag="mask")
            nc.sync.dma_start(mask_sb[:], mask)
            wq_sb = consts.tile([P, KO, DHL], mmdt, tag="wq")
            nc.sync.dma_start(wq_sb[:], wqT.rearrange("(ko ki) m -> ki ko m", ki=P))
            wk_sb = consts.tile([P, KO, DHL], mmdt, tag="wk")
            nc.sync.dma_start(wk_sb[:], wkT.rearrange("(ko ki) m -> ki ko m", ki=P))
            wv_sb = consts.tile([P, KO, DHL], mmdt, tag="wv")
            nc.sync.dma_start(wv_sb[:], wvT.rearrange("(ko ki) m -> ki ko m", ki=P))
            wo_sb = consts.tile([P, DHL // P, D], mmdt, tag="wo")
            nc.sync.dma_start(wo_sb[:], woT.rearrange("(ko ki) n -> ki ko n", ki=P))
            bias_sb = consts.tile([P, 6], f32, tag="bias")
            nc.sync.dma_start(bias_sb[:], bqkv.rearrange("t (o p) -> p (t o)", p=P))

            # ---- phase P: projections -------------------------------------
            qT_sb = big.tile([P, 2, S], mmdt, tag="qT")
            kT_sb = big.tile([P, 2, S], mmdt, tag="kT")
            v_sb = big.tile([P, S // P, DHL], pv_dt, tag="v")

            for t, (xT, w_sb) in enumerate(((xqT, wq_sb), (xkT, wk_sb), (xvT, wv_sb))):
                xT_r = xT.rearrange("(ko ki) s -> ki ko s", ki=P)
                for nch in range(S // CW):
                    x_t = xin.tile([P, KO, CW], mmdt, tag="x")
                    nc.sync.dma_start(x_t[:], xT_r[:, :, nch * CW:(nch + 1) * CW])
                    dst = (qT_sb, kT_sb, None)[t]
                    v_t = None if t < 2 else vtp.tile([P, 2, CW], f32, tag="vt")
                    for mb in range(2):
                        ps = ps512.tile([P, CW], f32, tag="ps")
                        for ko in range(KO):
                            nc.tensor.matmul(
                                ps[:],
                                w_sb[:, ko, mb * P:(mb + 1) * P],
                                x_t[:, ko, :],
                                start=(ko == 0),
                                stop=(ko == KO - 1),
                            )
                        tgt = (dst[:, mb, nch * CW:(nch + 1) * CW] if t < 2
                               else v_t[:, mb, :])
                        nc.vector.tensor_scalar_add(
                            tgt, ps[:], bias_sb[:, t * 2 + mb:t * 2 + mb + 1]
                        )
                    if t == 2:
                        # vT chunk -> v[S, dh] via grouped PE transposes.
                        for mb in range(2):
                            pst = pstr.tile([P, CW], f32, tag="pst")
                            for j in range(CW // P):
                                tr(pst[:, j * P:(j + 1) * P],
                                   v_t[:, mb, j * P:(j + 1) * P], ident_sb[:])
                            nc.any.tensor_copy(
                                v_sb[:, nch * 4:(nch + 1) * 4, mb * P:(mb + 1) * P],
                                pst[:].rearrange("p (a b) -> p a b", a=4),
                            )

            # ---- phase A: attention per local head ------------------------
            ctx_sb = big.tile([P, S // P, DHL], f32, tag="ctx")
            for h in range(HL):
                mb, off = h // 2, (h % 2) * DK
                for qb in range(S // P):
                    kw = (qb + 1) * P            # valid key width of this strip
                    p_t = pstrip.tile([P, S], f32, tag="p")
                    sums_t = small.tile([P, 8], f32, tag="sums")
                    ncch = (kw + CW - 1) // CW   # exact-width chunks
                    for c in range(ncch):
                        lo = c * CW
                        w = min(CW, kw - lo)
                        ps = ps512.tile([P, CW], f32, tag="ps")
                        nc.tensor.matmul(
                            ps[:, :w],
                            qT_sb[off:off + DK, mb, qb * P:(qb + 1) * P],
                            kT_sb[off:off + DK, mb, lo:lo + w],
                            start=True, stop=True,
                        )
                        if c == ncch - 1:
                            # additive causal mask on the diagonal block only
                            nc.vector.tensor_add(
                                ps[:, w - P:w], ps[:, w - P:w], mask_sb[:]
                            )
                        nc.scalar.activation(
                            p_t[:, lo:lo + w], ps[:, :w], Act.Exp,
                            scale=SCALE, accum_out=sums_t[:, c:c + 1],
                        )
                    nc.vector.reduce_sum(
                        out=sums_t[:, 7:8], in_=sums_t[:, 0:ncch],
                        axis=mybir.AxisListType.X,
                    )
                    nc.vector.reciprocal(sums_t[:, 6:7], sums_t[:, 7:8])
                    nc.vector.tensor_scalar_mul(
                        p_t[:, 0:kw], p_t[:, 0:kw], sums_t[:, 6:7]
                    )
                    nc.sync.dma_start(
                        p_out[h, qb * P:(qb + 1) * P, 0:kw], p_t[:, 0:kw]
                    )
                    # p @ v: grouped PE transposes of p, then fp16 matmuls.
                    psc = psctx.tile([P, DK], f32, tag="psc")
                    for g in range(ncch):
                        lo = g * CW
                        w = min(CW, kw - lo)
                        nb = w // P
                        pst = pstr.tile([P, CW], f32, tag="pst")
                        for j in range(nb):
                            tr(pst[:, j * P:(j + 1) * P],
                               p_t[:, lo + j * P:lo + (j + 1) * P], ident_sb[:])
                        pT_t = ptile.tile([P, CW], pv_dt, tag="pT")
                        nc.any.tensor_copy(pT_t[:, :w], pst[:, :w])
                        for j in range(nb):
                            kb = g * 4 + j
                            nc.tensor.matmul(
                                psc[:],
                                pT_t[:, j * P:(j + 1) * P],
                                v_sb[:, kb, h * DK:(h + 1) * DK],
                                start=(kb == 0), stop=(kb == qb),
                            )
                    nc.scalar.copy(ctx_sb[:, qb, h * DK:(h + 1) * DK], psc[:])

            # ---- phase O: output projection (partial over local dh) -------
            ctxT_sb = big.tile([P, DHL // P, S], mmdt, tag="ctxT")
            for qg in range(4):
                for mbd in range(DHL // P):
                    pst = pstr.tile([P, CW], f32, tag="pst")
                    for j in range(4):
                        tr(pst[:, j * P:(j + 1) * P],
                           ctx_sb[:, qg * 4 + j, mbd * P:(mbd + 1) * P], ident_sb[:])
                    nc.any.tensor_copy(
                        ctxT_sb[:, mbd, qg * CW:(qg + 1) * CW], pst[:]
                    )
            for sb in range(S // P):
                o_t = outp.tile([P, D], f32, tag="o")
                for nh in range(D // CW):
                    ps = ps512.tile([P, CW], f32, tag="ps")
                    for mbd in range(DHL // P):
                        nc.tensor.matmul(
                            ps[:],
                            ctxT_sb[:, mbd, sb * P:(sb + 1) * P],
                            wo_sb[:, mbd, nh * CW:(nh + 1) * CW],
                            start=(mbd == 0), stop=(mbd == DHL // P - 1),
                        )
                    nc.scalar.copy(o_t[:, nh * CW:(nh + 1) * CW], ps[:])
                nc.sync.dma_start(o_out[sb * P:(sb + 1) * P, :], o_t[:])

    nc.compile()
    return nc


def _get_nc():
    if "nc" not in _CACHE:
        _CACHE["nc"] = _build_nc()
    return _CACHE["nc"]


def _host_masks():
    # Additive causal mask for the 128x128 diagonal block.
    qr = np.arange(P)[:, None]
    j = np.arange(P)[None, :]
    return np.where(j <= qr, np.float32(0.0), np.float32(NEG))


def kernel(query, key, value, Wq, bq, Wk, bk, Wv, bv, Wo, bo):
    from concourse.bass_utils import run_bass_kernel_spmd

    nc = _get_nc()

    query = np.asarray(query, np.float32)
    key = np.asarray(key, np.float32)
    value = np.asarray(value, np.float32)
    WqT = np.ascontiguousarray(np.asarray(Wq, np.float32).T)
    WkT = np.ascontiguousarray(np.asarray(Wk, np.float32).T)
    WvT = np.ascontiguousarray(np.asarray(Wv, np.float32).T)
    WoT = np.ascontiguousarray(np.asarray(Wo, np.float32).T)
    bq = np.asarray(bq, np.float32)
    bk = np.asarray(bk, np.float32)
    bv = np.asarray(bv, np.float32)
    bo = np.asarray(bo, np.float32)

    mask = _host_masks()
    ident = np.eye(P, dtype=np.float32)

    in_maps = []
    for c in range(N_CORES):
        b, g = c // TP, c % TP
        cols = slice(g * DHL, (g + 1) * DHL)   # head dims owned by this core
        in_maps.append({
            "xqT": np.ascontiguousarray(query[b].T),
            "xkT": np.ascontiguousarray(key[b].T),
            "xvT": np.ascontiguousarray(value[b].T),
            "wqT": np.ascontiguousarray(WqT[:, cols]),
            "wkT": np.ascontiguousarray(WkT[:, cols]),
            "wvT": np.ascontiguousarray(WvT[:, cols]),
            "woT": np.ascontiguousarray(WoT[cols, :]),
            "bqkv": np.ascontiguousarray(np.stack([bq[cols], bk[cols], bv[cols]])),
            "mask": mask,
            "ident": ident,
        })

    res = run_bass_kernel_spmd(
        nc, in_maps, core_ids=list(range(N_CORES)),
        trace=bool(int(os.environ.get("KERNEL_TRACE", "0"))),
    )
    _CACHE["last_result"] = res

    p_attn = np.empty((B, H, S, S), np.float32)
    out = np.empty((B, S, D), np.float32)
    for b in range(B):
        acc = None
        for g in range(TP):
            rr = res.results[b * TP + g]
            p_attn[b, g * HL:(g + 1) * HL] = rr["p_out"]
            acc = rr["o_out"] if acc is None else acc + rr["o_out"]
        out[b] = acc + bo
    return out, p_attn


# revision 9
# speedup vs baseline: 2.3616x; 1.1200x over previous
"""Causal multi-head self-attention (B=2, S=2048, D=1024, H=16) for 8 trn2
NeuronCores.

Sharding: 2-way data parallel over batch x 4-way tensor parallel over heads.
Core c handles batch c//4 and heads [(c%4)*4, (c%4)*4+4) (dh_local=256).
Each core computes its heads' q/k/v projections, causal softmax attention
probabilities (written as the p_attn output shard), the attention context,
and a partial output projection over its 256 head-dims. The host sums the 4
partial output projections per batch (the TP all-reduce) and adds bo.

The upper-triangle (masked) region of p_attn is never written on device: the
runtime hands the kernel zero-initialised output buffers (both the native
run_neff path and the bass2jax/PJRT donation path guarantee this), so the
strictly-causal zeros come for free.

Device-side notes:
  - activations arrive pre-transposed (xT: [D, S]) so every matmul
    contraction dim lands on SBUF partitions.
  - q/k/v are produced transposed ([dh, S]); v is then PE-transposed back to
    [S, dh] (needed as the moving operand of p @ v) and stored fp16.
  - softmax needs no running-max pass: scores/sqrt(D) have |s| < ~1.
  - scores/projection/output matmuls run as float32r (full-rate PE); the
    softmax strip, its PE transposes, and the p @ v matmul run in fp16
    (p_out is written fp16 and widened to fp32 on the host). p @ v uses v
    as the stationary operand and 512-wide transposed-p chunks as the
    moving operand, producing the context directly transposed ([dh, S]).
"""

import os
import numpy as np

B, S, D, H = 2, 2048, 1024, 16
N_CORES = 8
TP = 4              # cores per batch (head-parallel group size)
HL = H // TP        # heads per core = 4
DK = D // H         # 64
DHL = HL * DK       # local head dims per core = 256
P = 128             # SBUF partitions
KO = D // P         # 8 contraction subtiles for the projections
CW = 512            # max chunk width (matmul moving-dim / PSUM bank)
NEG = -30000.0      # additive causal mask (exp -> exact 0 after /32 scale)
SCALE = 1.0 / float(np.sqrt(np.float32(D)))  # faithful sqrt(d_model) scale

F32R = bool(int(os.environ.get("KERNEL_F32R", "1")))
PV_F16 = bool(int(os.environ.get("KERNEL_PV_F16", "1")))

_CACHE = {}


def _build_nc():
    import concourse.bass as bass
    import concourse.mybir as mybir
    import concourse.tile as tile
    from concourse import bacc

    f32 = mybir.dt.float32
    f32r = mybir.dt.float32r
    f16 = mybir.dt.float16
    pv_dt = f16 if PV_F16 else f32
    mmdt = f32r if F32R else f32   # dtype of tensors feeding PE matmuls
    Act = mybir.ActivationFunctionType

    nc = bacc.Bacc(
        "TRN2",
        target_bir_lowering=False,
        debug=False,
        enable_asserts=False,
        num_devices=N_CORES,
    )

    # Per-core inputs.
    xqT = nc.dram_tensor("xqT", [D, S], mmdt, kind="ExternalInput").ap()
    xkT = nc.dram_tensor("xkT", [D, S], mmdt, kind="ExternalInput").ap()
    xvT = nc.dram_tensor("xvT", [D, S], mmdt, kind="ExternalInput").ap()
    wqT = nc.dram_tensor("wqT", [D, DHL], mmdt, kind="ExternalInput").ap()
    wkT = nc.dram_tensor("wkT", [D, DHL], mmdt, kind="ExternalInput").ap()
    wvT = nc.dram_tensor("wvT", [D, DHL], mmdt, kind="ExternalInput").ap()
    woT = nc.dram_tensor("woT", [DHL, D], mmdt, kind="ExternalInput").ap()
    bqkv = nc.dram_tensor("bqkv", [3, DHL], f32, kind="ExternalInput").ap()
    mask = nc.dram_tensor("mask", [P, P], f32, kind="ExternalInput").ap()
    ident = nc.dram_tensor("ident", [P, P], f16, kind="ExternalInput").ap()

    # Per-core outputs.
    p_out = nc.dram_tensor("p_out", [HL, S, S], f16, kind="ExternalOutput").ap()
    o_out = nc.dram_tensor("o_out", [S, D], f32, kind="ExternalOutput").ap()

    def tr(out_ap, in_ap, ident_ap):
        nc.tensor.transpose(out_ap, in_ap, ident_ap)

    with tile.TileContext(nc) as tc:
        with (
            tc.tile_pool(name="consts", bufs=1) as consts,
            tc.tile_pool(name="xin", bufs=2) as xin,
            tc.tile_pool(name="big", bufs=1) as big,
            tc.tile_pool(name="vt", bufs=2) as vtp,
            tc.tile_pool(name="pstrip", bufs=2) as pstrip,
            tc.tile_pool(name="ptile", bufs=2) as ptile,
            tc.tile_pool(name="outp", bufs=2) as outp,
            tc.tile_pool(name="small", bufs=4) as small,
            tc.tile_pool(name="ps512", bufs=3, space="PSUM") as ps512,
            tc.tile_pool(name="pstr", bufs=3, space="PSUM") as pstr,
            tc.tile_pool(name="psctx", bufs=2, space="PSUM") as psctx,
        ):
            # ---- constants -------------------------------------------------
            ident_sb = consts.tile([P, P], f16, tag="ident")
            nc.sync.dma_start(ident_sb[:], ident)
            mask_sb = consts.tile([P, P], f32, tag="mask")
            nc.sync.dma_start(mask_sb[:], mask)
            wq_sb = consts.tile([P, KO, DHL], mmdt, tag="wq")
            nc.sync.dma_start(wq_sb[:], wqT.rearrange("(ko ki) m -> ki ko m", ki=P))
            wk_sb = consts.tile([P, KO, DHL], mmdt, tag="wk")
            nc.sync.dma_start(wk_sb[:], wkT.rearrange("(ko ki) m -> ki ko m", ki=P))
            wv_sb = consts.tile([P, KO, DHL], mmdt, tag="wv")
            nc.sync.dma_start(wv_sb[:], wvT.rearrange("(ko ki) m -> ki ko m", ki=P))
            wo_sb = consts.tile([P, DHL // P, D], mmdt, tag="wo")
            nc.sync.dma_start(wo_sb[:], woT.rearrange("(ko ki) n -> ki ko n", ki=P))
            bias_sb = consts.tile([P, 6], f32, tag="bias")
            nc.sync.dma_start(bias_sb[:], bqkv.rearrange("t (o p) -> p (t o)", p=P))

            # ---- phase P: projections -------------------------------------
            qT_sb = big.tile([P, 2, S], mmdt, tag="qT")
            kT_sb = big.tile([P, 2, S], mmdt, tag="kT")
            v_sb = big.tile([P, S // P, DHL], pv_dt, tag="v")

            for t, (xT, w_sb) in enumerate(((xqT, wq_sb), (xkT, wk_sb), (xvT, wv_sb))):
                xT_r = xT.rearrange("(ko ki) s -> ki ko s", ki=P)
                for nch in range(S // CW):
                    x_t = xin.tile([P, KO, CW], mmdt, tag="x")
                    nc.sync.dma_start(x_t[:], xT_r[:, :, nch * CW:(nch + 1) * CW])
                    dst = (qT_sb, kT_sb, None)[t]
                    v_t = None if t < 2 else vtp.tile([P, 2, CW], pv_dt, tag="vt")
                    for mb in range(2):
                        ps = ps512.tile([P, CW], f32, tag="ps")
                        for ko in range(KO):
                            nc.tensor.matmul(
                                ps[:],
                                w_sb[:, ko, mb * P:(mb + 1) * P],
                                x_t[:, ko, :],
                                start=(ko == 0),
                                stop=(ko == KO - 1),
                            )
                        tgt = (dst[:, mb, nch * CW:(nch + 1) * CW] if t < 2
                               else v_t[:, mb, :])
                        nc.vector.tensor_scalar_add(
                            tgt, ps[:], bias_sb[:, t * 2 + mb:t * 2 + mb + 1]
                        )
                    if t == 2:
                        # vT chunk -> v[S, dh] via grouped PE transposes.
                        for mb in range(2):
                            pst = pstr.tile([P, CW], pv_dt, tag="pst")
                            for j in range(CW // P):
                                tr(pst[:, j * P:(j + 1) * P],
                                   v_t[:, mb, j * P:(j + 1) * P], ident_sb[:])
                            nc.any.tensor_copy(
                                v_sb[:, nch * 4:(nch + 1) * 4, mb * P:(mb + 1) * P],
                                pst[:].rearrange("p (a b) -> p a b", a=4),
                            )

            # ---- phase A: attention per local head ------------------------
            # ctxT is produced directly transposed: p @ v with v stationary.
            ctxT_sb = big.tile([P, DHL // P, S], mmdt, tag="ctxT")
            for h in range(HL):
                mb, off = h // 2, (h % 2) * DK
                for qg in range(4):
                    # transposed-p chunks for the 4 strips of this q-group:
                    # [key-in-block, kb, q-within-group], fp16.
                    nkb = 4 * qg + 4
                    pT_b = ptile.tile([P, S // P, CW], f16, tag="pT")
                    for j in range(3):
                        # blocks kb in (qb, nkb) stay zero for strip qb=4qg+j
                        qb = 4 * qg + j
                        nc.vector.memset(
                            pT_b[:, qb + 1:nkb, j * P:(j + 1) * P], 0.0
                        )
                    for j in range(4):
                        qb = 4 * qg + j
                        kw = (qb + 1) * P        # valid key width of the strip
                        p_t = pstrip.tile([P, S], f16, tag="p")
                        sums_t = small.tile([P, 8], f32, tag="sums")
                        ncch = (kw + CW - 1) // CW
                        for c in range(ncch):
                            lo = c * CW
                            w = min(CW, kw - lo)
                            ps = ps512.tile([P, CW], f32, tag="ps")
                            nc.tensor.matmul(
                                ps[:, :w],
                                qT_sb[off:off + DK, mb, qb * P:(qb + 1) * P],
                                kT_sb[off:off + DK, mb, lo:lo + w],
                                start=True, stop=True,
                            )
                            if c == ncch - 1:
                                nc.vector.tensor_add(
                                    ps[:, w - P:w], ps[:, w - P:w], mask_sb[:]
                                )
                            nc.scalar.activation(
                                p_t[:, lo:lo + w], ps[:, :w], Act.Exp,
                                scale=SCALE, accum_out=sums_t[:, c:c + 1],
                            )
                        nc.vector.reduce_sum(
                            out=sums_t[:, 7:8], in_=sums_t[:, 0:ncch],
                            axis=mybir.AxisListType.X,
                        )
                        nc.vector.reciprocal(sums_t[:, 6:7], sums_t[:, 7:8])
                        nc.any.tensor_scalar_mul(
                            p_t[:, 0:kw], p_t[:, 0:kw], sums_t[:, 6:7]
                        )
                        nc.sync.dma_start(
                            p_out[h, qb * P:(qb + 1) * P, 0:kw], p_t[:, 0:kw]
                        )
                        # transpose the strip into this q-group's pT chunks
                        for g in range(ncch):
                            lo = g * CW
                            w = min(CW, kw - lo)
                            nb = w // P
                            pst = pstr.tile([P, CW], f16, tag="pst")
                            for jj in range(nb):
                                tr(pst[:, jj * P:(jj + 1) * P],
                                   p_t[:, lo + jj * P:lo + (jj + 1) * P],
                                   ident_sb[:])
                            nc.any.tensor_copy(
                                pT_b[:, g * 4:g * 4 + nb, j * P:(j + 1) * P],
                                pst[:, :w].rearrange("p (a b) -> p a b", a=nb),
                            )
                    # p @ v for the whole q-group: ctxT[dk, 512q] accumulated
                    # over key blocks, v stationary / pT moving (512 cols).
                    psc = psctx.tile([DK, CW], f32, tag="psc")
                    for kb in range(nkb):
                        nc.tensor.matmul(
                            psc[:],
                            v_sb[:, kb, h * DK:(h + 1) * DK],
                            pT_b[:, kb, :],
                            start=(kb == 0), stop=(kb == nkb - 1),
                        )
                    nc.any.tensor_copy(
                        ctxT_sb[off:off + DK, mb, qg * CW:(qg + 1) * CW], psc[:]
                    )

            # ---- phase O: output projection (partial over local dh) -------
            for sb in range(S // P):
                o_t = outp.tile([P, D], f32, tag="o")
                for nh in range(D // CW):
                    ps = ps512.tile([P, CW], f32, tag="ps")
                    for mbd in range(DHL // P):
                        nc.tensor.matmul(
                            ps[:],
                            ctxT_sb[:, mbd, sb * P:(sb + 1) * P],
                            wo_sb[:, mbd, nh * CW:(nh + 1) * CW],
                            start=(mbd == 0), stop=(mbd == DHL // P - 1),
                        )
                    nc.scalar.copy(o_t[:, nh * CW:(nh + 1) * CW], ps[:])
                nc.sync.dma_start(o_out[sb * P:(sb + 1) * P, :], o_t[:])

    nc.compile()
    return nc


def _get_nc():
    if "nc" not in _CACHE:
        _CACHE["nc"] = _build_nc()
    return _CACHE["nc"]


def _host_masks():
    # Additive causal mask for the 128x128 diagonal block.
    qr = np.arange(P)[:, None]
    j = np.arange(P)[None, :]
    return np.where(j <= qr, np.float32(0.0), np.float32(NEG))


def kernel(query, key, value, Wq, bq, Wk, bk, Wv, bv, Wo, bo):
    from concourse.bass_utils import run_bass_kernel_spmd

    nc = _get_nc()

    query = np.asarray(query, np.float32)
    key = np.asarray(key, np.float32)
    value = np.asarray(value, np.float32)
    WqT = np.ascontiguousarray(np.asarray(Wq, np.float32).T)
    WkT = np.ascontiguousarray(np.asarray(Wk, np.float32).T)
    WvT = np.ascontiguousarray(np.asarray(Wv, np.float32).T)
    WoT = np.ascontiguousarray(np.asarray(Wo, np.float32).T)
    bq = np.asarray(bq, np.float32)
    bk = np.asarray(bk, np.float32)
    bv = np.asarray(bv, np.float32)
    bo = np.asarray(bo, np.float32)

    mask = _host_masks()
    ident = np.eye(P, dtype=np.float16)

    in_maps = []
    for c in range(N_CORES):
        b, g = c // TP, c % TP
        cols = slice(g * DHL, (g + 1) * DHL)   # head dims owned by this core
        in_maps.append({
            "xqT": np.ascontiguousarray(query[b].T),
            "xkT": np.ascontiguousarray(key[b].T),
            "xvT": np.ascontiguousarray(value[b].T),
            "wqT": np.ascontiguousarray(WqT[:, cols]),
            "wkT": np.ascontiguousarray(WkT[:, cols]),
            "wvT": np.ascontiguousarray(WvT[:, cols]),
            "woT": np.ascontiguousarray(WoT[cols, :]),
            "bqkv": np.ascontiguousarray(np.stack([bq[cols], bk[cols], bv[cols]])),
            "mask": mask,
            "ident": ident,
        })

    res = run_bass_kernel_spmd(
        nc, in_maps, core_ids=list(range(N_CORES)),
        trace=bool(int(os.environ.get("KERNEL_TRACE", "0"))),
    )
    _CACHE["last_result"] = res

    p_attn = np.empty((B, H, S, S), np.float32)
    out = np.empty((B, S, D), np.float32)
    for b in range(B):
        acc = None
        for g in range(TP):
            rr = res.results[b * TP + g]
            p_attn[b, g * HL:(g + 1) * HL] = rr["p_out"].astype(np.float32)
            acc = rr["o_out"] if acc is None else acc + rr["o_out"]
        out[b] = acc + bo
    return out, p_attn
